# revision 1
# baseline (speedup 1.0000x reference)
"""Trainium2 Bass kernel for nn_DilatedGraphConvolutionCell (8-core SPMD).

- Dead-code elimination: output = [Z0..Z3 at t=32] transitively needs only U
  columns {26..32,0,1}, conv at Z0:{28..32} Z1:{30,32} Z2:{32} Z3:{32}, and
  15 real FC rows + one shared fc(0) row.
- FC weights output-sharded 8 ways, fp32->fp16 cast-DMA, SBUF-resident;
  W-stationary matmuls give feature-on-partition outputs.
- Adjacency node-sharded; S computed transposed (softmax via ones-matmuls,
  no cross-partition reductions). A^T cached fp16 for all 25 pairs, reused by
  all 4 layers. Degree normalization (==1.0 +- 1e-7) skipped.
- All DMA patterns keep big partition strides + contiguous inner runs
  (transpose-style partition-stride-1 patterns are descriptor bombs).
  Where a transpose is unavoidable (conv Z -> next FC input) it's done on the
  PE via identity-matmul on [32,128] blocks.
"""
import numpy as np
from contextlib import ExitStack

import concourse.bass as bass
import concourse.tile as tile
from concourse import bacc, mybir
from concourse.bass_utils import run_bass_kernel_spmd
from concourse.masks import make_identity

F32 = mybir.dt.float32
F16 = mybir.dt.float16

NC = 8
N = 500
L = 33
FE = 128
DD = 64
DO = 64
FC1W = 1024
FC2W = 1024
KTF = 18000
NODES_PER_CORE = 64
REAL_NODES = [64] * 7 + [52]
NODE0 = [64 * c for c in range(NC)]

T9 = [26, 27, 28, 29, 30, 31, 32, 0, 1]
T9IDX = {t: i for i, t in enumerate(T9)}
T5 = [28, 29, 30, 31, 32]
PAIRS = []
PAIR_ID = {}
for _t in T5:
    for _d in range(-2, 3):
        _p = ((_t + _d) % L, _t)
        if _p not in PAIR_ID:
            PAIR_ID[_p] = len(PAIRS)
            PAIRS.append(_p)

CONV_TS = [[28, 29, 30, 31, 32], [30, 32], [32], [32]]
R_PASS = [8, 5, 2, 1]
XROW = {
    0: {t: (0, t - 26) for t in range(26, 33)},
    1: {t: (1, t - 28) for t in range(28, 33)},
    2: {30: (2, 0), 31: (0, 7), 32: (2, 1)},
    3: {30: (0, 7), 31: (0, 7), 32: (3, 0)},
}
MCH = [(0, 128), (128, 128), (256, 128), (384, 116)]
W3RES_J = 40
KT1 = 250
RG = [list(range(NC))]
RELU = mybir.ActivationFunctionType.Relu
EXP = mybir.ActivationFunctionType.Exp


def dap(handle, off, dims):
    """Custom AP: dims = [(step_elems, count), ...]; first dim = partitions."""
    t = handle.tensor if isinstance(handle, bass.AP) else handle
    base = handle.offset if isinstance(handle, bass.AP) else 0
    return bass.AP(tensor=t, offset=base + off, ap=[[s, n] for s, n in dims])


def rap(ap_obj, dims):
    """AP on same tensor as ap_obj with custom free dims (keeps partitions)."""
    return bass.AP(tensor=ap_obj.tensor, offset=ap_obj.offset,
                   ap=[list(ap_obj.ap[0])] + [[s, n] for s, n in dims])


def build(debug=False):
    nc = bacc.Bacc("TRN2", target_bir_lowering=False, debug=False,
                   num_devices=NC)

    def inp(name, shape):
        return nc.declare_dram_parameter(name, list(shape), F32, isOutput=False)

    li = inp("li", (N, NODES_PER_CORE, 9))     # host pre-T: [m, n_own, t]
    tfs = inp("tfs", (KTF // NC, 9))           # host pre-T: [k_own, t]
    obs7t = inp("obs7t", (128, 7, 32))         # host pre-T, own 32 kt
    ws1 = inp("ws1", (N, 256))
    bs1 = inp("bs1", (256,))
    ws2 = inp("ws2", (256, FE))
    bs2 = inp("bs2", (FE,))
    wt1s = inp("wt1s", (KTF // NC, 256))
    bt1 = inp("bt1", (256,))
    wt2 = inp("wt2", (256, FE))
    bt2 = inp("bt2", (FE,))
    bmat = inp("bmat", (FE, FE))
    w1s = inp("w1s", (4096, FC1W))             # row-shard (k-sharded FC1)
    b1st = inp("b1st", (1, FC1W))              # b1/8 bias row
    w2s = inp("w2s", (FC1W, FC2W))             # full (replicated FC2)
    b2st = inp("b2st", (1, FC2W))              # b2 bias row
    w3s = inp("w3s", (FC2W, 8192))
    b3st = inp("b3st", (1, 8192))              # b3 bias row (padded)
    wfb = inp("wfb", (5, FE, DO))
    bconv = inp("bconv", (DO,))

    out_ext = nc.declare_dram_parameter(
        "out", [4, NODES_PER_CORE, DO], F32, isOutput=True)
    dbg = {}
    if debug:
        dbg["dbg_u"] = nc.declare_dram_parameter(
            "dbg_u", [NC * 128, 576], F32, isOutput=True)
        dbg["dbg_x0"] = nc.declare_dram_parameter(
            "dbg_x0", [NC * 128, 8 * 64], F32, isOutput=True)
        dbg["dbg_h1"] = nc.declare_dram_parameter(
            "dbg_h1", [FC1W, 8], F32, isOutput=True)
        dbg["dbg_at"] = nc.declare_dram_parameter(
            "dbg_at", [128, 25 * 4 * 64], F32, isOutput=True)

    with ExitStack() as ctx:
        tc = ctx.enter_context(tile.TileContext(nc))
        pw = ctx.enter_context(tc.tile_pool(name="pw", bufs=1))
        dram = ctx.enter_context(tc.tile_pool(name="dram", bufs=1, space="DRAM"))

        ones_c = pw.tile([128, 1], F32)
        nc.vector.memset(ones_c, 1.0)
        ones_r = pw.tile([1, 128], F32)
        nc.vector.memset(ones_r, 1.0)
        ident = pw.tile([128, 128], F16)
        make_identity(nc, ident)
        b1r_sb = pw.tile([1, 8, 128], F16)
        nc.gpsimd.dma_start(out=b1r_sb,
                            in_=dap(b1st, 0, [(0, 1), (128, 8), (1, 128)]))
        b2r_sb = pw.tile([1, 8, 128], F16)
        nc.gpsimd.dma_start(out=b2r_sb,
                            in_=dap(b2st, 0, [(0, 1), (128, 8), (1, 128)]))
        ones16 = pw.tile([1, 8], F16)
        nc.vector.memset(ones16, 1.0)
        ones64 = pw.tile([1, 64], F16)
        nc.vector.memset(ones64, 1.0)
        bcr_sb = pw.tile([1, 64], F16)
        nc.gpsimd.dma_start(out=bcr_sb, in_=dap(bconv, 0, [(0, 1), (1, 64)]))
        bcb_sb = pw.tile([64, 64], F32)        # bconv broadcast over nodes
        nc.gpsimd.dma_start(out=bcb_sb, in_=dap(bconv, 0, [(0, 64), (1, 64)]))
        wfb_sb = pw.tile([128, 5, 64], F16)
        nc.gpsimd.dma_start(
            out=wfb_sb, in_=dap(wfb, 0, [(64, 128), (128 * 64, 5), (1, 64)]))

        w1_sb = pw.tile([128, 32, 8, 128], F16)   # [k%128, kt_own, ct, col]
        nc.gpsimd.dma_start(
            out=w1_sb,
            in_=dap(w1s, 0, [(1024, 128), (128 * 1024, 32), (128, 8), (1, 128)]))
        w2_sb = pw.tile([128, 8, 8, 128], F16)    # [k%128, kt, ct_out, col]
        nc.gpsimd.dma_start(
            out=w2_sb,
            in_=dap(w2s, 0, [(1024, 128), (128 * 1024, 8), (128, 8), (1, 128)]))

        at_sb = pw.tile([128, 25, 4, 64], F16)
        zrow_sb = pw.tile([128, 8, 64], F16)

        # =============== U phase + adjacency ===============
        with tc.tile_pool(name="pu", bufs=1) as pu, \
             tc.tile_pool(name="pue", bufs=6) as pue, \
             tc.tile_pool(name="ppsu", bufs=1, space="PSUM") as ppsu:
            liT = pu.tile([128, 4, 64, 9], F32)
            nc.gpsimd.dma_start(
                out=liT[:125].rearrange("p mt n t -> p mt (n t)"),
                in_=dap(li, 0, [(576, 125), (125 * 576, 4), (1, 576)]))
            ws1_sb = pu.tile([128, 4, 2, 128], F32)
            for mt in range(4):
                nc.gpsimd.dma_start(
                    out=ws1_sb[:125, mt],
                    in_=dap(ws1, mt * 125 * 256,
                            [(256, 125), (128, 2), (1, 128)]))
            ws2_sb = pu.tile([128, 2, 128], F32)
            nc.gpsimd.dma_start(
                out=ws2_sb, in_=dap(ws2, 0, [(128, 128), (128 * 128, 2), (1, 128)]))
            bs1_sb = pu.tile([128, 2], F32)
            nc.gpsimd.dma_start(out=bs1_sb, in_=dap(bs1, 0, [(1, 128), (128, 2)]))
            bs2_sb = pu.tile([128, 1], F32)
            nc.gpsimd.dma_start(out=bs2_sb, in_=dap(bs2, 0, [(1, 128), (0, 1)]))
            b_sb = pu.tile([128, 128], F32)
            nc.gpsimd.dma_start(out=b_sb, in_=dap(bmat, 0, [(128, 128), (1, 128)]))
            tfT = pu.tile([128, 18, 9], F32)
            nc.gpsimd.dma_start(
                out=tfT[:125],
                in_=dap(tfs, 0, [(9, 125), (125 * 9, 18), (1, 9)]))
            wt1_sb = pu.tile([128, 18, 2, 128], F32)
            for kt in range(18):
                nc.gpsimd.dma_start(
                    out=wt1_sb[:125, kt],
                    in_=dap(wt1s, kt * 125 * 256,
                            [(256, 125), (128, 2), (1, 128)]))
            bt1_sb = pu.tile([128, 2], F32)
            nc.gpsimd.dma_start(out=bt1_sb, in_=dap(bt1, 0, [(1, 128), (128, 2)]))
            wt2_sb = pu.tile([128, 2, 128], F32)
            nc.gpsimd.dma_start(
                out=wt2_sb, in_=dap(wt2, 0, [(128, 128), (128 * 128, 2), (1, 128)]))
            bt2_sb = pu.tile([128, 1], F32)
            nc.gpsimd.dma_start(out=bt2_sb, in_=dap(bt2, 0, [(1, 128), (0, 1)]))

            # temporal MLP layer 1 partial + AllReduce
            ut1p = pu.tile([128, 2, 9], F32)
            for ct in range(2):
                ps = ppsu.tile([128, 9], F32, tag="ut", bufs=1)
                for kt in range(18):
                    nc.tensor.matmul(ps, wt1_sb[:125, kt, ct, :],
                                     tfT[:125, kt, :],
                                     start=(kt == 0), stop=(kt == 17))
                nc.vector.tensor_copy(ut1p[:, ct, :], ps)
            ut1i = dram.tile([256, 9], F32, tag="ut1i")
            ut1o = dram.tile([256, 9], F32, tag="ut1o", addr_space="Shared")
            nc.sync.dma_start(
                out=dap(ut1i, 0, [(9, 128), (128 * 9, 2), (1, 9)]), in_=ut1p)
            nc.gpsimd.collective_compute(
                "AllReduce", mybir.AluOpType.add, replica_groups=RG,
                ins=[ut1i.opt()], outs=[ut1o.opt()])
            ut1r = pu.tile([128, 2, 9], F32)
            nc.sync.dma_start(
                out=ut1r, in_=dap(ut1o, 0, [(9, 128), (128 * 9, 2), (1, 9)]))
            ut1a = pu.tile([128, 2, 9], F32)
            for ct in range(2):
                nc.scalar.activation(ut1a[:, ct, :], ut1r[:, ct, :], RELU,
                                     bias=bt1_sb[:, ct:ct + 1])
            utT = pu.tile([128, 9], F32)
            psu = ppsu.tile([128, 9], F32, tag="ut", bufs=1)
            for ct in range(2):
                nc.tensor.matmul(psu, wt2_sb[:, ct, :], ut1a[:, ct, :],
                                 start=(ct == 0), stop=(ct == 1))
            nc.scalar.activation(utT, psu, RELU, bias=bt2_sb)

            # spatial MLP (own nodes)
            us1T = pu.tile([128, 2, 576], F32)
            rhs_li = liT[:125].rearrange("p mt n t -> p mt (n t)")
            for ct in range(2):
                for ch in range(2):
                    ps = ppsu.tile([128, 288], F32, tag="us", bufs=1)
                    for mt in range(4):
                        nc.tensor.matmul(
                            ps, ws1_sb[:125, mt, ct, :],
                            rhs_li[:, mt, 288 * ch:288 * (ch + 1)],
                            start=(mt == 0), stop=(mt == 3))
                    nc.scalar.activation(us1T[:, ct, 288 * ch:288 * (ch + 1)],
                                         ps, RELU, bias=bs1_sb[:, ct:ct + 1])
            usT = pu.tile([128, 576], F32)
            for ch in range(2):
                ps = ppsu.tile([128, 288], F32, tag="us", bufs=1)
                for ct in range(2):
                    nc.tensor.matmul(ps, ws2_sb[:, ct, :],
                                     us1T[:, ct, 288 * ch:288 * (ch + 1)],
                                     start=(ct == 0), stop=(ct == 1))
                nc.scalar.activation(usT[:, 288 * ch:288 * (ch + 1)], ps, RELU,
                                     bias=bs2_sb)
            uT_own = pu.tile([128, 9, 64], F32)   # t-major: [f, t, n]
            usT_v = usT.rearrange("p (n t) -> p n t", t=9)
            for ti in range(9):
                nc.vector.tensor_scalar(
                    out=uT_own[:, ti, :], in0=usT_v[:, :, ti],
                    scalar1=utT[:, ti:ti + 1], scalar2=None,
                    op0=mybir.AluOpType.add)

            # U AllGather (f-major: rows (c,f), cols (t,n))
            uagi = dram.tile([128, 576], F32, tag="uagi")
            uago = dram.tile([NC * 128, 576], F32, tag="uago",
                             addr_space="Shared")
            nc.sync.dma_start(
                out=dap(uagi, 0, [(576, 128), (64, 9), (1, 64)]),
                in_=uT_own)
            nc.gpsimd.collective_compute(
                "AllGather", mybir.AluOpType.bypass, replica_groups=RG,
                ins=[uagi.opt()], outs=[uago.opt()])
            if debug:
                nc.sync.dma_start(out=dbg["dbg_u"][:, :], in_=uago[:, :])
            u2_sb = pu.tile([128, 5, 512], F32)
            for ti in range(5):
                nc.sync.dma_start(
                    out=u2_sb[:, ti, :].rearrange("p (c n) -> p c n", c=8),
                    in_=dap(uago, (2 + ti) * 64,
                            [(576, 128), (128 * 576, 8), (1, 64)]))

            # UB^T own
            ubT = pu.tile([128, 9, 64], F32)
            for ti in range(9):
                ps = ppsu.tile([128, 64], F32, tag="ub", bufs=1)
                nc.tensor.matmul(ps, b_sb, uT_own[:, ti, :],
                                 start=True, stop=True)
                nc.vector.tensor_copy(ubT[:, ti, :], ps)

            # adjacency pairs
            for (i1, i2) in PAIRS:
                pid = PAIR_ID[(i1, i2)]
                ti1 = T9IDX[i1]
                t2 = i2 - 28
                sps = ppsu.tile([128, 4, 64], F32, tag="spair", bufs=3)
                for mc, (m0, cnt) in enumerate(MCH):
                    nc.tensor.matmul(sps[:cnt, mc, :],
                                     u2_sb[:, t2, m0:m0 + cnt],
                                     ubT[:, ti1, :], start=True, stop=True)
                msk = pue.tile([128, 4, 64], F32, tag="msk")
                e_sb = pue.tile([128, 4, 64], F32, tag="esb")
                for mc, (m0, cnt) in enumerate(MCH):
                    nc.vector.tensor_scalar(
                        out=msk[:cnt, mc, :], in0=sps[:cnt, mc, :],
                        scalar1=0.05, scalar2=None, op0=mybir.AluOpType.is_ge)
                    nc.vector.tensor_tensor(
                        out=msk[:cnt, mc, :], in0=msk[:cnt, mc, :],
                        in1=sps[:cnt, mc, :], op=mybir.AluOpType.mult)
                    nc.scalar.activation(e_sb[:cnt, mc, :], msk[:cnt, mc, :],
                                         EXP)
                cps = ppsu.tile([1, 64], F32, tag="cs", bufs=1)
                for mc, (m0, cnt) in enumerate(MCH):
                    nc.tensor.matmul(cps, ones_c[:cnt, :], e_sb[:cnt, mc, :],
                                     start=(mc == 0), stop=(mc == 3))
                rcp = pue.tile([1, 64], F32, tag="rcp")
                nc.vector.reciprocal(rcp, cps)
                rbp = ppsu.tile([128, 64], F32, tag="rb", bufs=1)
                nc.tensor.matmul(rbp, ones_r, rcp, start=True, stop=True)
                for mc, (m0, cnt) in enumerate(MCH):
                    nc.vector.tensor_tensor(
                        out=at_sb[:cnt, pid, mc, :], in0=e_sb[:cnt, mc, :],
                        in1=rbp[:cnt, :], op=mybir.AluOpType.mult)
            if debug:
                at32 = pue.tile([128, 25 * 4 * 64], F32, tag="at32", bufs=1)
                nc.vector.tensor_copy(
                    at32, at_sb.rearrange("p a b c -> p (a b c)"))
                nc.sync.dma_start(out=dbg["dbg_at"][:, :], in_=at32)

        # =============== FC3 resident weights ===============
        pw3 = ctx.enter_context(tc.tile_pool(name="pw3", bufs=1))
        w3_sb = pw3.tile([128, 8, W3RES_J, 128], F16)
        for sl in range(4):
            j0 = sl * 10
            nc.gpsimd.dma_start(
                out=w3_sb[:, :, j0:j0 + 10, :],
                in_=dap(w3s, j0 * 128,
                        [(8192, 128), (128 * 8192, 8), (128, 10), (1, 128)]))

        # =============== passes ===============
        ptr = ctx.enter_context(tc.tile_pool(name="ptr", bufs=2))
        pps = ctx.enter_context(tc.tile_pool(name="pps", bufs=1, space="PSUM"))

        h1ri = [dram.tile([FC1W, R_PASS[p]], F32, name=f"h1ri{p}",
                          tag=f"h1ri{p}") for p in range(4)]
        h1ro = [dram.tile([FC1W, R_PASS[p]], F32, name=f"h1ro{p}",
                          tag=f"h1ro{p}", addr_space="Shared")
                for p in range(4)]
        # X AllGather: f-major rows (c,f), cols (r,j)
        xagi = [dram.tile([128, R_PASS[p] * 64], F32, name=f"xagi{p}",
                          tag=f"xagi{p}") for p in range(4)]
        xago = [dram.tile([NC * 128, R_PASS[p] * 64], F32, name=f"xago{p}",
                          tag=f"xago{p}", addr_space="Shared")
                for p in range(4)]
        xT_next = None
        for p in range(4):
            R = R_PASS[p]
            # ---- x^T own tiles [k%128, r, kt_own(32)] ----
            if p == 0:
                xT = ptr.tile([128, 8, 32], F16, tag="xTnA", bufs=1)
                nc.gpsimd.dma_start(
                    out=xT[:, :7, :],
                    in_=dap(obs7t, 0, [(7 * 32, 128), (32, 7), (1, 32)]))
                nc.vector.memset(xT[:, 7, :], 0.0)
            else:
                xT = xT_next            # built during previous pass's conv
            # ---- FC1 (k-sharded, partial sums over own 32 k-tiles) ----
            ps1 = pps.tile([128, 8, 8], F32, tag="fcA", bufs=1,
                           name=f"ps1_{p}")
            for ct in range(8):
                for kt in range(32):
                    nc.tensor.matmul(ps1[:, ct, :R], w1_sb[:, kt, ct, :],
                                     xT[:, :R, kt],
                                     start=(kt == 0), stop=False)
                nc.tensor.matmul(ps1[:, ct, :R], b1r_sb[:, ct, :],
                                 ones16[:, :R], start=False, stop=True)
            h1p = ptr.tile([128, 8, 8], F32, tag="h1p", bufs=1, name=f"h1p{p}")
            nc.vector.tensor_copy(h1p[:, :, :R], ps1[:, :, :R])
            nc.sync.dma_start(
                out=dap(h1ri[p], 0, [(R, 128), (128 * R, 8), (1, R)]),
                in_=h1p[:, :, :R])
            nc.gpsimd.collective_compute(
                "AllReduce", mybir.AluOpType.add, replica_groups=RG,
                ins=[h1ri[p].opt()], outs=[h1ro[p].opt()])
            h1f = ptr.tile([128, 8, 8], F32, tag="h1f", bufs=1, name=f"h1f{p}")
            nc.sync.dma_start(
                out=h1f[:, :, :R],
                in_=dap(h1ro[p], 0, [(R, 128), (128 * R, 8), (1, R)]))
            h1T = ptr.tile([128, 8, 8], F16, tag="h1T", name=f"h1T{p}")
            nc.scalar.activation(h1T[:, :, :R], h1f[:, :, :R], RELU)
            if debug and p == 0:
                nc.sync.dma_start(out=dbg["dbg_h1"][:, :], in_=h1ro[0][:, :])
            # ---- FC2 (replicated, local) ----
            ps2 = pps.tile([128, 8, 8], F32, tag="fcA", bufs=1,
                           name=f"ps2_{p}")
            h2T = ptr.tile([128, 8, 8], F16, tag="h2T", name=f"h2T{p}")
            for cto in range(8):
                for kt in range(8):
                    nc.tensor.matmul(ps2[:, cto, :R], w2_sb[:, kt, cto, :],
                                     h1T[:, kt, :R],
                                     start=(kt == 0), stop=False)
                nc.tensor.matmul(ps2[:, cto, :R], b2r_sb[:, cto, :],
                                 ones16[:, :R], start=False, stop=True)
            nc.scalar.activation(h2T[:, :, :R], ps2[:, :, :R], RELU)
            # ---- FC3 ----
            xstage = ptr.tile([128, 8, 64], F32, tag="xstage", bufs=1,
                              name=f"xstage{p}")   # [f, r, j]
            w3t = None
            for jg in range(8):
                b3g = ptr.tile([1, 8, 128], F16, tag="b3g", bufs=2,
                               name=f"b3g_{p}_{jg}")
                nc.gpsimd.dma_start(
                    out=b3g,
                    in_=dap(b3st, jg * 1024, [(0, 1), (128, 8), (1, 128)]))
                ps3 = pps.tile([128, 8, 8], F32, tag="fc3", bufs=2,
                               name=f"ps3_{p}_{jg}")
                for jj in range(8):
                    j = jg * 8 + jj
                    if j < W3RES_J:
                        wblk = lambda kt, j=j: w3_sb[:, kt, j, :]
                    else:
                        if (j - W3RES_J) % 2 == 0:
                            w3t = ptr.tile([128, 8, 2, 128], F16, tag="w3t",
                                           name=f"w3t{p}_{j}")
                            nc.gpsimd.dma_start(
                                out=w3t,
                                in_=dap(w3s, j * 128,
                                        [(8192, 128), (128 * 8192, 8),
                                         (128, 2), (1, 128)]))
                        wblk = (lambda kt, j=j, w3t=w3t:
                                w3t[:, kt, (j - W3RES_J) % 2, :])
                    for kt in range(8):
                        nc.tensor.matmul(ps3[:, jj, :R], wblk(kt),
                                         h2T[:, kt, :R],
                                         start=(kt == 0), stop=False)
                    nc.tensor.matmul(ps3[:, jj, :R], b3g[:, jj, :],
                                     ones16[:, :R], start=False, stop=True)
                nc.scalar.activation(
                    out=xstage[:, :R, jg * 8:(jg + 1) * 8],
                    in_=rap(ps3, [(1, R), (8, 8)]),
                    func=RELU)
            # ---- X AllGather (f-major) ----
            nc.sync.dma_start(
                out=dap(xagi[p], 0, [(R * 64, 128), (64, R), (1, 64)]),
                in_=xstage[:, :R, :])
            nc.gpsimd.collective_compute(
                "AllGather", mybir.AluOpType.bypass, replica_groups=RG,
                ins=[xagi[p].opt()], outs=[xago[p].opt()])
            if debug and p == 0:
                nc.sync.dma_start(out=dbg["dbg_x0"][:, :], in_=xago[0][:, :])

            # ---- Xk^T tiles [f, c, n] ----
            need = sorted({(t - k) % L for t in CONV_TS[p] for k in range(3)})
            xkT = {}
            for tv in need:
                sp, slot = XROW[p][tv]
                if sp == 0 and slot == 7 and p != 0:
                    xkT[tv] = zrow_sb
                    continue
                dst = ptr.tile([128, 8, 64], F16, tag=f"xk{tv % 4}",
                               name=f"xk_{p}_{tv}")
                Rs = R_PASS[sp]
                nc.gpsimd.dma_start(
                    out=dst,
                    in_=dap(xago[sp], slot * 64,
                            [(Rs * 64, 128), (128 * Rs * 64, 8), (1, 64)]))
                xkT[tv] = dst
            if p == 0:
                nc.gpsimd.dma_start(
                    out=zrow_sb,
                    in_=dap(xago[0], 7 * 64,
                            [(8 * 64, 128), (128 * 8 * 64, 8), (1, 64)]))

            # ---- conv units ----
            if p < 3:
                xT_next = ptr.tile([128, 8, 32], F16,
                                   tag=f"xTn{'B' if p % 2 == 0 else 'A'}",
                                   bufs=1, name=f"xTn{p + 1}")
            for t in CONV_TS[p]:
                relu_parts = []
                for k in range(3):
                    tv = (t - k) % L
                    xk = xkT[tv].rearrange("p c n -> p (c n)")
                    if k == 0:
                        dirs = [(PAIR_ID[(t, t)], 0, 64)]
                        wsl = wfb_sb[:, 0, :]
                        ow = 64
                    else:
                        dirs = [(PAIR_ID[((t - k) % L, t)], 0, 64),
                                (PAIR_ID[((t + k) % L, t)], 64, 128)]
                        wsl = wfb_sb.rearrange("p d o -> p (d o)")[
                            :, (2 * k - 1) * 64:(2 * k + 1) * 64]
                        ow = 128
                    y_ps = pps.tile([128, 4, 128], F32, tag="yps", bufs=2,
                                    name=f"yps{p}_{t}_{k}")
                    y_sb = ptr.tile([128, 4, 128], F16, tag="ysb",
                                    name=f"ysb{p}_{t}_{k}")
                    for mc, (m0, cnt) in enumerate(MCH):
                        nc.tensor.matmul(y_ps[:cnt, mc, :ow],
                                         xk[:, m0:m0 + cnt],
                                         wsl, start=True, stop=True)
                    nc.vector.tensor_copy(y_sb[:, :, :ow], y_ps[:, :, :ow])
                    hps = pps.tile([64, 64], F32, tag="hps", bufs=2,
                                   name=f"hps{p}_{t}_{k}")
                    for mc, (m0, cnt) in enumerate(MCH):
                        for (pid, o0, o1) in dirs:
                            nc.tensor.matmul(
                                hps, at_sb[:cnt, pid, mc, :],
                                y_sb[:cnt, mc, o0:o1],
                                start=(mc == 0 and o0 == 0), stop=False)
                    nc.tensor.matmul(hps, ones64[:, :64], bcr_sb,
                                     start=False, stop=True)
                    rk = ptr.tile([64, 64], F32, tag=f"rk{k}",
                                  name=f"rk{p}_{t}_{k}")
                    nc.scalar.activation(rk, hps, RELU)
                    relu_parts.append(rk)
                zacc = ptr.tile([64, 64], F32, tag="zacc",
                                name=f"zacc{p}_{t}")
                nc.vector.tensor_tensor(out=zacc, in0=relu_parts[0],
                                        in1=relu_parts[1],
                                        op=mybir.AluOpType.add)
                nc.vector.tensor_tensor(out=zacc, in0=zacc,
                                        in1=relu_parts[2],
                                        op=mybir.AluOpType.add)
                if t == 32:
                    nc.sync.dma_start(
                        out=dap(out_ext, p * NODES_PER_CORE * DO,
                                [(64, 64), (1, 64)]),
                        in_=zacc)
                if p < 3:
                    # feed next pass's x^T (own nodes only -- no Z exchange)
                    r = CONV_TS[p].index(t)
                    zc16 = ptr.tile([64, 64], F16, tag="zc16", bufs=1,
                                    name=f"zc16_{p}_{t}")
                    nc.vector.tensor_copy(zc16, zacc)
                    tpz = pps.tile([64, 64], F16, tag="tp", bufs=1,
                                   name=f"tpz{p}_{t}")
                    nc.tensor.transpose(tpz, zc16, ident[:64, :64])
                    # tpz = zacc^T [o, n]; node-pair j -> partitions (o, o+64)
                    nc.vector.tensor_copy(xT_next[0:64, r, :],
                                          rap(tpz, [(2, 32)]))
                    nc.vector.tensor_copy(xT_next[64:128, r, :],
                                          dap(tpz, 1, [list(tpz.ap[0])[0:2],
                                                       (2, 32)]) if False else
                                          bass.AP(tensor=tpz.tensor,
                                                  offset=tpz.offset + 1,
                                                  ap=[list(tpz.ap[0]),
                                                      [2, 32]]))

    nc.finalize()
    return nc


# ======================= host side =======================
_NC_CACHE = {}


def _get_nc(debug=False):
    if debug not in _NC_CACHE:
        _NC_CACHE[debug] = build(debug)
    return _NC_CACHE[debug]


def make_in_maps(inputs):
    obs = np.asarray(inputs["observation"], np.float32)
    tf = np.asarray(inputs["time_feats"], np.float32)
    lin = np.asarray(inputs["layer_initial"], np.float32)
    Wf = np.asarray(inputs["Wf"], np.float32)
    Wb = np.asarray(inputs["Wb"], np.float32)
    wfb = np.ascontiguousarray(
        np.stack([Wf[0] + Wb[0], Wf[1], Wb[1], Wf[2], Wb[2]]))
    w3 = np.asarray(inputs["Wfc3"], np.float32)
    b3 = np.asarray(inputs["bfc3"], np.float32)
    o7 = obs[26:33]                                   # (7, 32000)
    o7p = np.zeros((7, 256, 128), np.float32)
    o7p[:, :KT1, :] = o7.reshape(7, KT1, 128)
    obs7t_full = o7p.transpose(2, 0, 1)               # (128, 7, 256)
    w1full = np.asarray(inputs["Wfc1"], np.float32)   # (32000, 1024)
    b1 = np.asarray(inputs["bfc1"], np.float32)
    b2 = np.asarray(inputs["bfc2"], np.float32)
    w2full = np.ascontiguousarray(np.asarray(inputs["Wfc2"], np.float32))
    b1st = np.ascontiguousarray((b1 / NC).reshape(1, -1))
    b2st = np.ascontiguousarray(b2.reshape(1, -1))
    kfn = KTF // NC
    in_maps = []
    for c in range(NC):
        n0, cnt = NODE0[c], REAL_NODES[c]
        li = np.zeros((N, NODES_PER_CORE, 9), np.float32)
        li[:, :cnt, :] = lin[n0:n0 + cnt][:, T9, :].transpose(2, 0, 1)
        w3s = np.zeros((FC2W, 8192), np.float32)
        b3s = np.zeros((8192,), np.float32)
        c0, c1 = 8192 * c, min(8192 * (c + 1), 64000)
        w3s[:, :c1 - c0] = w3[:, c0:c1]
        b3s[:c1 - c0] = b3[c0:c1]
        b3st = np.ascontiguousarray(b3s.reshape(1, -1))
        w1row = np.zeros((4096, 1024), np.float32)
        k0, k1 = 4096 * c, min(4096 * (c + 1), 32000)
        w1row[:k1 - k0] = w1full[k0:k1]
        kf0 = kfn * c
        in_maps.append({
            "li": li,
            "tfs": np.ascontiguousarray(tf[T9][:, kf0:kf0 + kfn].T),
            "obs7t": np.ascontiguousarray(obs7t_full[:, :, 32 * c:32 * (c + 1)]),
            "ws1": np.asarray(inputs["Ws1"], np.float32),
            "bs1": np.asarray(inputs["bs1"], np.float32),
            "ws2": np.asarray(inputs["Ws2"], np.float32),
            "bs2": np.asarray(inputs["bs2"], np.float32),
            "wt1s": np.ascontiguousarray(
                np.asarray(inputs["Wt1"], np.float32)[kf0:kf0 + kfn]),
            "bt1": np.asarray(inputs["bt1"], np.float32),
            "wt2": np.asarray(inputs["Wt2"], np.float32),
            "bt2": np.asarray(inputs["bt2"], np.float32),
            "bmat": np.asarray(inputs["B"], np.float32),
            "w1s": w1row,
            "b1st": b1st,
            "w2s": w2full,
            "b2st": b2st,
            "w3s": w3s,
            "b3st": b3st,
            "wfb": wfb,
            "bconv": np.asarray(inputs["bconv"], np.float32),
        })
    return in_maps


def _assemble(results):
    out = np.zeros((4, N, DO), np.float32)
    for c in range(NC):
        n0, cnt = NODE0[c], REAL_NODES[c]
        out[:, n0:n0 + cnt, :] = results[c]["out"][:, :cnt, :]
    return out


def kernel(**inputs):
    nc = _get_nc(debug=False)
    in_maps = make_in_maps(inputs)
    res = run_bass_kernel_spmd(nc, in_maps, core_ids=list(range(NC)))
    return _assemble(res.results)



# revision 8
# speedup vs baseline: 1.4416x; 1.4416x over previous
"""Trainium2 Bass kernel for nn_DilatedGraphConvolutionCell (8-core SPMD).

- Dead-code elimination: output = [Z0..Z3 at t=32] transitively needs only U
  columns {26..32,0,1}, conv at Z0:{28..32} Z1:{30,32} Z2:{32} Z3:{32}, and
  15 real FC rows + one shared fc(0) row.
- FC weights output-sharded 8 ways, fp32->fp16 cast-DMA, SBUF-resident;
  W-stationary matmuls give feature-on-partition outputs.
- Adjacency node-sharded; S computed transposed (softmax via ones-matmuls,
  no cross-partition reductions). A^T cached fp16 for all 25 pairs, reused by
  all 4 layers. Degree normalization (==1.0 +- 1e-7) skipped.
- All DMA patterns keep big partition strides + contiguous inner runs
  (transpose-style partition-stride-1 patterns are descriptor bombs).
  Where a transpose is unavoidable (conv Z -> next FC input) it's done on the
  PE via identity-matmul on [32,128] blocks.
"""
import os
import numpy as np
from contextlib import ExitStack

ABLATE_CC = bool(int(os.environ.get("ABLATE_CC", "0")))

import concourse.bass as bass
import concourse.tile as tile
from concourse import bacc, mybir
from concourse.bass_utils import run_bass_kernel_spmd
from concourse.masks import make_identity

F32 = mybir.dt.float32
F16 = mybir.dt.float16

NC = 8
N = 500
L = 33
FE = 128
DD = 64
DO = 64
FC1W = 1024
FC2W = 1024
KTF = 18000
NODES_PER_CORE = 64
REAL_NODES = [64] * 7 + [52]
NODE0 = [64 * c for c in range(NC)]

T9 = [26, 27, 28, 29, 30, 31, 32, 0, 1]
T9IDX = {t: i for i, t in enumerate(T9)}
T5 = [28, 29, 30, 31, 32]
PAIRS = []
PAIR_ID = {}
for _t in T5:
    for _d in range(-2, 3):
        _p = ((_t + _d) % L, _t)
        if _p not in PAIR_ID:
            PAIR_ID[_p] = len(PAIRS)
            PAIRS.append(_p)

CONV_TS = [[28, 29, 30, 31, 32], [30, 32], [32], [32]]
R_PASS = [8, 5, 2, 1]
XROW = {
    0: {t: (0, t - 26) for t in range(26, 33)},
    1: {t: (1, t - 28) for t in range(28, 33)},
    2: {30: (2, 0), 31: (0, 7), 32: (2, 1)},
    3: {30: (0, 7), 31: (0, 7), 32: (3, 0)},
}
MCH = [(0, 128), (128, 128), (256, 128), (384, 116)]
W3RES_J = 40
KT1 = 250
RG = [list(range(NC))]
RELU = mybir.ActivationFunctionType.Relu
EXP = mybir.ActivationFunctionType.Exp


def dap(handle, off, dims):
    """Custom AP: dims = [(step_elems, count), ...]; first dim = partitions."""
    t = handle.tensor if isinstance(handle, bass.AP) else handle
    base = handle.offset if isinstance(handle, bass.AP) else 0
    return bass.AP(tensor=t, offset=base + off, ap=[[s, n] for s, n in dims])


def rap(ap_obj, dims):
    """AP on same tensor as ap_obj with custom free dims (keeps partitions)."""
    return bass.AP(tensor=ap_obj.tensor, offset=ap_obj.offset,
                   ap=[list(ap_obj.ap[0])] + [[s, n] for s, n in dims])


def build(debug=False):
    nc = bacc.Bacc("TRN2", target_bir_lowering=False, debug=False,
                   num_devices=NC)

    def inp(name, shape):
        return nc.declare_dram_parameter(name, list(shape), F32, isOutput=False)

    li = inp("li", (N, NODES_PER_CORE, 9))     # host pre-T: [m, n_own, t]
    tfs = inp("tfs", (KTF // NC, 9))           # host pre-T: [k_own, t]
    obs7t = inp("obs7t", (128, 7, 32))         # host pre-T, own 32 kt
    ws1 = inp("ws1", (N, 256))
    bs1 = inp("bs1", (256,))
    ws2 = inp("ws2", (256, FE))
    bs2 = inp("bs2", (FE,))
    wt1s = inp("wt1s", (KTF // NC, 256))
    bt1 = inp("bt1", (256,))
    wt2 = inp("wt2", (256, FE))
    bt2 = inp("bt2", (FE,))
    bmat = inp("bmat", (FE, FE))
    w1s = inp("w1s", (4096, FC1W))             # row-shard (k-sharded FC1)
    b1st = inp("b1st", (1, FC1W))              # b1/8 bias row
    w2s = inp("w2s", (FC1W, FC2W))             # full (replicated FC2)
    b2st = inp("b2st", (1, FC2W))              # b2 bias row
    w3s = inp("w3s", (FC2W, 8192))
    b3st = inp("b3st", (1, 8192))              # b3 bias row (padded)
    wfb = inp("wfb", (5, FE, DO))
    bconv = inp("bconv", (DO,))

    out_ext = nc.declare_dram_parameter(
        "out", [4, NODES_PER_CORE, DO], F32, isOutput=True)
    dbg = {}
    if debug:
        dbg["dbg_u"] = nc.declare_dram_parameter(
            "dbg_u", [NC * 128, 576], F32, isOutput=True)
        dbg["dbg_x0"] = nc.declare_dram_parameter(
            "dbg_x0", [NC * 128, 8 * 64], F32, isOutput=True)
        dbg["dbg_h1"] = nc.declare_dram_parameter(
            "dbg_h1", [FC1W, 8], F32, isOutput=True)
        dbg["dbg_at"] = nc.declare_dram_parameter(
            "dbg_at", [128, 25 * 4 * 64], F32, isOutput=True)

    def cc(kind, alu, ins_t, outs_t):
        if ABLATE_CC:
            nc.sync.dma_start(out=outs_t[:ins_t.shape[0]], in_=ins_t[:])
        else:
            nc.gpsimd.collective_compute(
                kind, alu, replica_groups=RG,
                ins=[ins_t.opt()], outs=[outs_t.opt()])

    with ExitStack() as ctx:
        tc = ctx.enter_context(tile.TileContext(nc))
        pw = ctx.enter_context(tc.tile_pool(name="pw", bufs=1))
        dram = ctx.enter_context(tc.tile_pool(name="dram", bufs=1, space="DRAM"))

        ones_c = pw.tile([128, 1], F32)
        nc.vector.memset(ones_c, 1.0)
        ones_r = pw.tile([1, 128], F32)
        nc.vector.memset(ones_r, 1.0)
        ident = pw.tile([128, 128], F16)
        make_identity(nc, ident)
        b1r_sb = pw.tile([1, 8, 128], F16)
        nc.gpsimd.dma_start(out=b1r_sb,
                            in_=dap(b1st, 0, [(0, 1), (128, 8), (1, 128)]))
        b2r_sb = pw.tile([1, 8, 128], F16)
        nc.gpsimd.dma_start(out=b2r_sb,
                            in_=dap(b2st, 0, [(0, 1), (128, 8), (1, 128)]))
        ones16 = pw.tile([1, 8], F16)
        nc.vector.memset(ones16, 1.0)
        ones64 = pw.tile([1, 64], F16)
        nc.vector.memset(ones64, 1.0)
        bcr_sb = pw.tile([1, 64], F16)
        nc.gpsimd.dma_start(out=bcr_sb, in_=dap(bconv, 0, [(0, 1), (1, 64)]))
        bcb_sb = pw.tile([64, 64], F32)        # bconv broadcast over nodes
        nc.gpsimd.dma_start(out=bcb_sb, in_=dap(bconv, 0, [(0, 64), (1, 64)]))
        wfb_sb = pw.tile([128, 5, 64], F16)
        nc.gpsimd.dma_start(
            out=wfb_sb, in_=dap(wfb, 0, [(64, 128), (128 * 64, 5), (1, 64)]))

        w1_sb = pw.tile([128, 32, 8, 128], F16)   # [k%128, kt_own, ct, col]
        nc.gpsimd.dma_start(
            out=w1_sb,
            in_=dap(w1s, 0, [(1024, 128), (128 * 1024, 32), (128, 8), (1, 128)]))
        w2_sb = pw.tile([128, 8, 8, 128], F16)    # [k%128, kt, ct_out, col]
        nc.gpsimd.dma_start(
            out=w2_sb,
            in_=dap(w2s, 0, [(1024, 128), (128 * 1024, 8), (128, 8), (1, 128)]))

        at_sb = pw.tile([128, 25, 4, 64], F16)
        zrow_sb = pw.tile([128, 8, 64], F16)

        # =============== U phase + adjacency ===============
        with tc.tile_pool(name="pu", bufs=1) as pu, \
             tc.tile_pool(name="pue", bufs=6) as pue, \
             tc.tile_pool(name="ppsu", bufs=1, space="PSUM") as ppsu:
            liT = pu.tile([128, 4, 64, 9], F32)
            nc.gpsimd.dma_start(
                out=liT[:125].rearrange("p mt n t -> p mt (n t)"),
                in_=dap(li, 0, [(576, 125), (125 * 576, 4), (1, 576)]))
            ws1_sb = pu.tile([128, 4, 2, 128], F32)
            for mt in range(4):
                nc.gpsimd.dma_start(
                    out=ws1_sb[:125, mt],
                    in_=dap(ws1, mt * 125 * 256,
                            [(256, 125), (128, 2), (1, 128)]))
            ws2_sb = pu.tile([128, 2, 128], F32)
            nc.gpsimd.dma_start(
                out=ws2_sb, in_=dap(ws2, 0, [(128, 128), (128 * 128, 2), (1, 128)]))
            bs1_sb = pu.tile([128, 2], F32)
            nc.gpsimd.dma_start(out=bs1_sb, in_=dap(bs1, 0, [(1, 128), (128, 2)]))
            bs2_sb = pu.tile([128, 1], F32)
            nc.gpsimd.dma_start(out=bs2_sb, in_=dap(bs2, 0, [(1, 128), (0, 1)]))
            b_sb = pu.tile([128, 128], F32)
            nc.gpsimd.dma_start(out=b_sb, in_=dap(bmat, 0, [(128, 128), (1, 128)]))
            tfT = pu.tile([128, 18, 9], F32)
            nc.gpsimd.dma_start(
                out=tfT[:125],
                in_=dap(tfs, 0, [(9, 125), (125 * 9, 18), (1, 9)]))
            wt1_sb = pu.tile([128, 18, 2, 128], F32)
            for kt in range(18):
                nc.gpsimd.dma_start(
                    out=wt1_sb[:125, kt],
                    in_=dap(wt1s, kt * 125 * 256,
                            [(256, 125), (128, 2), (1, 128)]))
            bt1_sb = pu.tile([128, 2], F32)
            nc.gpsimd.dma_start(out=bt1_sb, in_=dap(bt1, 0, [(1, 128), (128, 2)]))
            wt2_sb = pu.tile([128, 2, 128], F32)
            nc.gpsimd.dma_start(
                out=wt2_sb, in_=dap(wt2, 0, [(128, 128), (128 * 128, 2), (1, 128)]))
            bt2_sb = pu.tile([128, 1], F32)
            nc.gpsimd.dma_start(out=bt2_sb, in_=dap(bt2, 0, [(1, 128), (0, 1)]))

            # temporal MLP layer 1 partial + AllReduce
            ut1p = pu.tile([128, 2, 9], F32)
            for ct in range(2):
                ps = ppsu.tile([128, 9], F32, tag="ut", bufs=1)
                for kt in range(18):
                    nc.tensor.matmul(ps, wt1_sb[:125, kt, ct, :],
                                     tfT[:125, kt, :],
                                     start=(kt == 0), stop=(kt == 17))
                nc.vector.tensor_copy(ut1p[:, ct, :], ps)
            ut1i = dram.tile([256, 9], F32, tag="ut1i")
            ut1o = dram.tile([256, 9], F32, tag="ut1o", addr_space="Shared")
            nc.sync.dma_start(
                out=dap(ut1i, 0, [(9, 128), (128 * 9, 2), (1, 9)]), in_=ut1p)
            cc("AllReduce", mybir.AluOpType.add, ut1i, ut1o)
            ut1r = pu.tile([128, 2, 9], F32)
            nc.sync.dma_start(
                out=ut1r, in_=dap(ut1o, 0, [(9, 128), (128 * 9, 2), (1, 9)]))
            ut1a = pu.tile([128, 2, 9], F32)
            for ct in range(2):
                nc.scalar.activation(ut1a[:, ct, :], ut1r[:, ct, :], RELU,
                                     bias=bt1_sb[:, ct:ct + 1])
            utT = pu.tile([128, 9], F32)
            psu = ppsu.tile([128, 9], F32, tag="ut", bufs=1)
            for ct in range(2):
                nc.tensor.matmul(psu, wt2_sb[:, ct, :], ut1a[:, ct, :],
                                 start=(ct == 0), stop=(ct == 1))
            nc.scalar.activation(utT, psu, RELU, bias=bt2_sb)

            # spatial MLP (own nodes)
            us1T = pu.tile([128, 2, 576], F32)
            rhs_li = liT[:125].rearrange("p mt n t -> p mt (n t)")
            for ct in range(2):
                for ch in range(2):
                    ps = ppsu.tile([128, 288], F32, tag="us", bufs=1)
                    for mt in range(4):
                        nc.tensor.matmul(
                            ps, ws1_sb[:125, mt, ct, :],
                            rhs_li[:, mt, 288 * ch:288 * (ch + 1)],
                            start=(mt == 0), stop=(mt == 3))
                    nc.scalar.activation(us1T[:, ct, 288 * ch:288 * (ch + 1)],
                                         ps, RELU, bias=bs1_sb[:, ct:ct + 1])
            usT = pu.tile([128, 576], F32)
            for ch in range(2):
                ps = ppsu.tile([128, 288], F32, tag="us", bufs=1)
                for ct in range(2):
                    nc.tensor.matmul(ps, ws2_sb[:, ct, :],
                                     us1T[:, ct, 288 * ch:288 * (ch + 1)],
                                     start=(ct == 0), stop=(ct == 1))
                nc.scalar.activation(usT[:, 288 * ch:288 * (ch + 1)], ps, RELU,
                                     bias=bs2_sb)
            uT_own = pu.tile([128, 9, 64], F32)   # t-major: [f, t, n]
            usT_v = usT.rearrange("p (n t) -> p n t", t=9)
            for ti in range(9):
                nc.vector.tensor_scalar(
                    out=uT_own[:, ti, :], in0=usT_v[:, :, ti],
                    scalar1=utT[:, ti:ti + 1], scalar2=None,
                    op0=mybir.AluOpType.add)

            # U AllGather (f-major: rows (c,f), cols (t,n))
            uagi = dram.tile([128, 576], F32, tag="uagi")
            uago = dram.tile([NC * 128, 576], F32, tag="uago",
                             addr_space="Shared")
            nc.sync.dma_start(
                out=dap(uagi, 0, [(576, 128), (64, 9), (1, 64)]),
                in_=uT_own)
            cc("AllGather", mybir.AluOpType.bypass, uagi, uago)
            if debug:
                nc.sync.dma_start(out=dbg["dbg_u"][:, :], in_=uago[:, :])
            u2_sb = pu.tile([128, 5, 512], F32)
            for ti in range(5):
                nc.sync.dma_start(
                    out=u2_sb[:, ti, :].rearrange("p (c n) -> p c n", c=8),
                    in_=dap(uago, (2 + ti) * 64,
                            [(576, 128), (128 * 576, 8), (1, 64)]))

            # UB^T own
            ubT = pu.tile([128, 9, 64], F32)
            for ti in range(9):
                ps = ppsu.tile([128, 64], F32, tag="ub", bufs=1)
                nc.tensor.matmul(ps, b_sb, uT_own[:, ti, :],
                                 start=True, stop=True)
                nc.vector.tensor_copy(ubT[:, ti, :], ps)

            # adjacency pairs
            for (i1, i2) in PAIRS:
                pid = PAIR_ID[(i1, i2)]
                ti1 = T9IDX[i1]
                t2 = i2 - 28
                sps = ppsu.tile([128, 4, 64], F32, tag="spair", bufs=3)
                for mc, (m0, cnt) in enumerate(MCH):
                    nc.tensor.matmul(sps[:cnt, mc, :],
                                     u2_sb[:, t2, m0:m0 + cnt],
                                     ubT[:, ti1, :], start=True, stop=True)
                msk = pue.tile([128, 4, 64], F32, tag="msk")
                e_sb = pue.tile([128, 4, 64], F32, tag="esb")
                for mc, (m0, cnt) in enumerate(MCH):
                    nc.vector.tensor_scalar(
                        out=msk[:cnt, mc, :], in0=sps[:cnt, mc, :],
                        scalar1=0.05, scalar2=None, op0=mybir.AluOpType.is_ge)
                    nc.vector.tensor_tensor(
                        out=msk[:cnt, mc, :], in0=msk[:cnt, mc, :],
                        in1=sps[:cnt, mc, :], op=mybir.AluOpType.mult)
                    nc.scalar.activation(e_sb[:cnt, mc, :], msk[:cnt, mc, :],
                                         EXP)
                cps = ppsu.tile([1, 64], F32, tag="cs", bufs=1)
                for mc, (m0, cnt) in enumerate(MCH):
                    nc.tensor.matmul(cps, ones_c[:cnt, :], e_sb[:cnt, mc, :],
                                     start=(mc == 0), stop=(mc == 3))
                rcp = pue.tile([1, 64], F32, tag="rcp")
                nc.vector.reciprocal(rcp, cps)
                rbp = ppsu.tile([128, 64], F32, tag="rb", bufs=1)
                nc.tensor.matmul(rbp, ones_r, rcp, start=True, stop=True)
                for mc, (m0, cnt) in enumerate(MCH):
                    nc.vector.tensor_tensor(
                        out=at_sb[:cnt, pid, mc, :], in0=e_sb[:cnt, mc, :],
                        in1=rbp[:cnt, :], op=mybir.AluOpType.mult)
            if debug:
                at32 = pue.tile([128, 25 * 4 * 64], F32, tag="at32", bufs=1)
                nc.vector.tensor_copy(
                    at32, at_sb.rearrange("p a b c -> p (a b c)"))
                nc.sync.dma_start(out=dbg["dbg_at"][:, :], in_=at32)

        # =============== FC3 resident weights ===============
        pw3 = ctx.enter_context(tc.tile_pool(name="pw3", bufs=1))
        w3_sb = pw3.tile([128, 8, W3RES_J, 128], F16)
        for sl in range(4):
            j0 = sl * 10
            nc.gpsimd.dma_start(
                out=w3_sb[:, :, j0:j0 + 10, :],
                in_=dap(w3s, j0 * 128,
                        [(8192, 128), (128 * 8192, 8), (128, 10), (1, 128)]))

        # =============== passes ===============
        ptr = ctx.enter_context(tc.tile_pool(name="ptr", bufs=2))
        pps = ctx.enter_context(tc.tile_pool(name="pps", bufs=1, space="PSUM"))

        h1ri = [dram.tile([FC1W, R_PASS[p]], F32, name=f"h1ri{p}",
                          tag=f"h1ri{p}") for p in range(4)]
        h1ro = [dram.tile([FC1W, R_PASS[p]], F32, name=f"h1ro{p}",
                          tag=f"h1ro{p}", addr_space="Shared")
                for p in range(4)]
        # X AllGather: f-major rows (c,f), cols (r,j)
        xagi = [dram.tile([128, R_PASS[p] * 64], F32, name=f"xagi{p}",
                          tag=f"xagi{p}") for p in range(4)]
        xago = [dram.tile([NC * 128, R_PASS[p] * 64], F32, name=f"xago{p}",
                          tag=f"xago{p}", addr_space="Shared")
                for p in range(4)]
        xT_next = None
        for p in range(4):
            R = R_PASS[p]
            # ---- x^T own tiles [k%128, r, kt_own(32)] ----
            if p == 0:
                xT = ptr.tile([128, 8, 32], F16, tag="xTnA", bufs=1)
                nc.gpsimd.dma_start(
                    out=xT[:, :7, :],
                    in_=dap(obs7t, 0, [(7 * 32, 128), (32, 7), (1, 32)]))
                nc.vector.memset(xT[:, 7, :], 0.0)
            else:
                xT = xT_next            # built during previous pass's conv
            # ---- FC1 (k-sharded, partial sums over own 32 k-tiles) ----
            ps1 = pps.tile([128, 8, 8], F32, tag="fcA", bufs=1,
                           name=f"ps1_{p}")
            for ct in range(8):
                for kt in range(32):
                    nc.tensor.matmul(ps1[:, ct, :R], w1_sb[:, kt, ct, :],
                                     xT[:, :R, kt],
                                     start=(kt == 0), stop=False)
                nc.tensor.matmul(ps1[:, ct, :R], b1r_sb[:, ct, :],
                                 ones16[:, :R], start=False, stop=True)
            h1p = ptr.tile([128, 8, 8], F32, tag="h1p", bufs=1, name=f"h1p{p}")
            nc.vector.tensor_copy(h1p[:, :, :R], ps1[:, :, :R])
            nc.sync.dma_start(
                out=dap(h1ri[p], 0, [(R, 128), (128 * R, 8), (1, R)]),
                in_=h1p[:, :, :R])
            cc("AllReduce", mybir.AluOpType.add, h1ri[p], h1ro[p])
            h1f = ptr.tile([128, 8, 8], F32, tag="h1f", bufs=1, name=f"h1f{p}")
            nc.sync.dma_start(
                out=h1f[:, :, :R],
                in_=dap(h1ro[p], 0, [(R, 128), (128 * R, 8), (1, R)]))
            h1T = ptr.tile([128, 8, 8], F16, tag="h1T", name=f"h1T{p}")
            nc.scalar.activation(h1T[:, :, :R], h1f[:, :, :R], RELU)
            if debug and p == 0:
                nc.sync.dma_start(out=dbg["dbg_h1"][:, :], in_=h1ro[0][:, :])
            # ---- FC2 (replicated, local) ----
            ps2 = pps.tile([128, 8, 8], F32, tag="fcA", bufs=1,
                           name=f"ps2_{p}")
            h2T = ptr.tile([128, 8, 8], F16, tag="h2T", name=f"h2T{p}")
            for cto in range(8):
                for kt in range(8):
                    nc.tensor.matmul(ps2[:, cto, :R], w2_sb[:, kt, cto, :],
                                     h1T[:, kt, :R],
                                     start=(kt == 0), stop=False)
                nc.tensor.matmul(ps2[:, cto, :R], b2r_sb[:, cto, :],
                                 ones16[:, :R], start=False, stop=True)
            nc.scalar.activation(h2T[:, :, :R], ps2[:, :, :R], RELU)
            # ---- FC3 ----
            xstage = ptr.tile([128, 8, 64], F32, tag="xstage", bufs=1,
                              name=f"xstage{p}")   # [f, r, j]
            w3t = None
            for jg in range(8):
                b3g = ptr.tile([1, 8, 128], F16, tag="b3g", bufs=2,
                               name=f"b3g_{p}_{jg}")
                nc.gpsimd.dma_start(
                    out=b3g,
                    in_=dap(b3st, jg * 1024, [(0, 1), (128, 8), (1, 128)]))
                ps3 = pps.tile([128, 8, 8], F32, tag="fc3", bufs=2,
                               name=f"ps3_{p}_{jg}")
                for jj in range(8):
                    j = jg * 8 + jj
                    if j < W3RES_J:
                        wblk = lambda kt, j=j: w3_sb[:, kt, j, :]
                    else:
                        if (j - W3RES_J) % 2 == 0:
                            w3t = ptr.tile([128, 8, 2, 128], F16, tag="w3t",
                                           name=f"w3t{p}_{j}")
                            nc.gpsimd.dma_start(
                                out=w3t,
                                in_=dap(w3s, j * 128,
                                        [(8192, 128), (128 * 8192, 8),
                                         (128, 2), (1, 128)]))
                        wblk = (lambda kt, j=j, w3t=w3t:
                                w3t[:, kt, (j - W3RES_J) % 2, :])
                    for kt in range(8):
                        nc.tensor.matmul(ps3[:, jj, :R], wblk(kt),
                                         h2T[:, kt, :R],
                                         start=(kt == 0), stop=False)
                    nc.tensor.matmul(ps3[:, jj, :R], b3g[:, jj, :],
                                     ones16[:, :R], start=False, stop=True)
                nc.scalar.activation(
                    out=xstage[:, :R, jg * 8:(jg + 1) * 8],
                    in_=rap(ps3, [(1, R), (8, 8)]),
                    func=RELU)
            # ---- X AllGather (f-major) ----
            nc.sync.dma_start(
                out=dap(xagi[p], 0, [(R * 64, 128), (64, R), (1, 64)]),
                in_=xstage[:, :R, :])
            cc("AllGather", mybir.AluOpType.bypass, xagi[p], xago[p])
            if debug and p == 0:
                nc.sync.dma_start(out=dbg["dbg_x0"][:, :], in_=xago[0][:, :])

            # ---- Xk^T tiles [f, c, n] ----
            need = sorted({(t - k) % L for t in CONV_TS[p] for k in range(3)})
            xkT = {}
            for tv in need:
                sp, slot = XROW[p][tv]
                if sp == 0 and slot == 7 and p != 0:
                    xkT[tv] = zrow_sb
                    continue
                dst = ptr.tile([128, 8, 64], F16, tag=f"xk{tv % 4}",
                               name=f"xk_{p}_{tv}")
                Rs = R_PASS[sp]
                nc.gpsimd.dma_start(
                    out=dst,
                    in_=dap(xago[sp], slot * 64,
                            [(Rs * 64, 128), (128 * Rs * 64, 8), (1, 64)]))
                xkT[tv] = dst
            if p == 0:
                nc.gpsimd.dma_start(
                    out=zrow_sb,
                    in_=dap(xago[0], 7 * 64,
                            [(8 * 64, 128), (128 * 8 * 64, 8), (1, 64)]))

            # ---- conv units ----
            if p < 3:
                xT_next = ptr.tile([128, 8, 32], F16,
                                   tag=f"xTn{'B' if p % 2 == 0 else 'A'}",
                                   bufs=1, name=f"xTn{p + 1}")
            for t in CONV_TS[p]:
                relu_parts = []
                for k in range(3):
                    tv = (t - k) % L
                    xk = xkT[tv].rearrange("p c n -> p (c n)")
                    if k == 0:
                        dirs = [(PAIR_ID[(t, t)], 0, 64)]
                        wsl = wfb_sb[:, 0, :]
                        ow = 64
                    else:
                        dirs = [(PAIR_ID[((t - k) % L, t)], 0, 64),
                                (PAIR_ID[((t + k) % L, t)], 64, 128)]
                        wsl = wfb_sb.rearrange("p d o -> p (d o)")[
                            :, (2 * k - 1) * 64:(2 * k + 1) * 64]
                        ow = 128
                    y_ps = pps.tile([128, 4, 128], F32, tag="yps", bufs=2,
                                    name=f"yps{p}_{t}_{k}")
                    y_sb = ptr.tile([128, 4, 128], F16, tag="ysb",
                                    name=f"ysb{p}_{t}_{k}")
                    for mc, (m0, cnt) in enumerate(MCH):
                        nc.tensor.matmul(y_ps[:cnt, mc, :ow],
                                         xk[:, m0:m0 + cnt],
                                         wsl, start=True, stop=True)
                    for mc, (m0, cnt) in enumerate(MCH):
                        nc.vector.tensor_copy(y_sb[:cnt, mc, :ow],
                                              y_ps[:cnt, mc, :ow])
                    hps = pps.tile([64, 64], F32, tag="hps", bufs=2,
                                   name=f"hps{p}_{t}_{k}")
                    for mc, (m0, cnt) in enumerate(MCH):
                        for (pid, o0, o1) in dirs:
                            nc.tensor.matmul(
                                hps, at_sb[:cnt, pid, mc, :],
                                y_sb[:cnt, mc, o0:o1],
                                start=(mc == 0 and o0 == 0), stop=False)
                    nc.tensor.matmul(hps, ones64[:, :64], bcr_sb,
                                     start=False, stop=True)
                    rk = ptr.tile([64, 64], F32, tag=f"rk{k}",
                                  name=f"rk{p}_{t}_{k}")
                    nc.scalar.activation(rk, hps, RELU)
                    relu_parts.append(rk)
                zacc = ptr.tile([64, 64], F32, tag="zacc",
                                name=f"zacc{p}_{t}")
                nc.vector.tensor_tensor(out=zacc, in0=relu_parts[0],
                                        in1=relu_parts[1],
                                        op=mybir.AluOpType.add)
                nc.vector.tensor_tensor(out=zacc, in0=zacc,
                                        in1=relu_parts[2],
                                        op=mybir.AluOpType.add)
                if t == 32:
                    nc.sync.dma_start(
                        out=dap(out_ext, p * NODES_PER_CORE * DO,
                                [(64, 64), (1, 64)]),
                        in_=zacc)
                if p < 3:
                    # feed next pass's x^T (own nodes only -- no Z exchange)
                    r = CONV_TS[p].index(t)
                    zc16 = ptr.tile([64, 64], F16, tag="zc16", bufs=1,
                                    name=f"zc16_{p}_{t}")
                    nc.vector.tensor_copy(zc16, zacc)
                    tpz = pps.tile([64, 64], F16, tag="tp", bufs=1,
                                   name=f"tpz{p}_{t}")
                    nc.tensor.transpose(tpz, zc16, ident[:64, :64])
                    # tpz = zacc^T [o, n]; node-pair j -> partitions (o, o+64)
                    nc.vector.tensor_copy(xT_next[0:64, r, :],
                                          rap(tpz, [(2, 32)]))
                    nc.vector.tensor_copy(xT_next[64:128, r, :],
                                          dap(tpz, 1, [list(tpz.ap[0])[0:2],
                                                       (2, 32)]) if False else
                                          bass.AP(tensor=tpz.tensor,
                                                  offset=tpz.offset + 1,
                                                  ap=[list(tpz.ap[0]),
                                                      [2, 32]]))

    nc.finalize()
    return nc


# ======================= host side =======================
_NC_CACHE = {}


def _get_nc(debug=False):
    if debug not in _NC_CACHE:
        _NC_CACHE[debug] = build(debug)
    return _NC_CACHE[debug]


def make_in_maps(inputs):
    obs = np.asarray(inputs["observation"], np.float32)
    tf = np.asarray(inputs["time_feats"], np.float32)
    lin = np.asarray(inputs["layer_initial"], np.float32)
    Wf = np.asarray(inputs["Wf"], np.float32)
    Wb = np.asarray(inputs["Wb"], np.float32)
    wfb = np.ascontiguousarray(
        np.stack([Wf[0] + Wb[0], Wf[1], Wb[1], Wf[2], Wb[2]]))
    w3 = np.asarray(inputs["Wfc3"], np.float32)
    b3 = np.asarray(inputs["bfc3"], np.float32)
    o7 = obs[26:33]                                   # (7, 32000)
    o7p = np.zeros((7, 256, 128), np.float32)
    o7p[:, :KT1, :] = o7.reshape(7, KT1, 128)
    obs7t_full = o7p.transpose(2, 0, 1)               # (128, 7, 256)
    w1full = np.asarray(inputs["Wfc1"], np.float32)   # (32000, 1024)
    b1 = np.asarray(inputs["bfc1"], np.float32)
    b2 = np.asarray(inputs["bfc2"], np.float32)
    w2full = np.ascontiguousarray(np.asarray(inputs["Wfc2"], np.float32))
    b1st = np.ascontiguousarray((b1 / NC).reshape(1, -1))
    b2st = np.ascontiguousarray(b2.reshape(1, -1))
    kfn = KTF // NC
    in_maps = []
    for c in range(NC):
        n0, cnt = NODE0[c], REAL_NODES[c]
        li = np.zeros((N, NODES_PER_CORE, 9), np.float32)
        li[:, :cnt, :] = lin[n0:n0 + cnt][:, T9, :].transpose(2, 0, 1)
        w3s = np.zeros((FC2W, 8192), np.float32)
        b3s = np.zeros((8192,), np.float32)
        c0, c1 = 8192 * c, min(8192 * (c + 1), 64000)
        w3s[:, :c1 - c0] = w3[:, c0:c1]
        b3s[:c1 - c0] = b3[c0:c1]
        b3st = np.ascontiguousarray(b3s.reshape(1, -1))
        w1row = np.zeros((4096, 1024), np.float32)
        k0, k1 = 4096 * c, min(4096 * (c + 1), 32000)
        w1row[:k1 - k0] = w1full[k0:k1]
        kf0 = kfn * c
        in_maps.append({
            "li": li,
            "tfs": np.ascontiguousarray(tf[T9][:, kf0:kf0 + kfn].T),
            "obs7t": np.ascontiguousarray(obs7t_full[:, :, 32 * c:32 * (c + 1)]),
            "ws1": np.asarray(inputs["Ws1"], np.float32),
            "bs1": np.asarray(inputs["bs1"], np.float32),
            "ws2": np.asarray(inputs["Ws2"], np.float32),
            "bs2": np.asarray(inputs["bs2"], np.float32),
            "wt1s": np.ascontiguousarray(
                np.asarray(inputs["Wt1"], np.float32)[kf0:kf0 + kfn]),
            "bt1": np.asarray(inputs["bt1"], np.float32),
            "wt2": np.asarray(inputs["Wt2"], np.float32),
            "bt2": np.asarray(inputs["bt2"], np.float32),
            "bmat": np.asarray(inputs["B"], np.float32),
            "w1s": w1row,
            "b1st": b1st,
            "w2s": w2full,
            "b2st": b2st,
            "w3s": w3s,
            "b3st": b3st,
            "wfb": wfb,
            "bconv": np.asarray(inputs["bconv"], np.float32),
        })
    return in_maps


def _assemble(results):
    out = np.zeros((4, N, DO), np.float32)
    for c in range(NC):
        n0, cnt = NODE0[c], REAL_NODES[c]
        out[:, n0:n0 + cnt, :] = results[c]["out"][:, :cnt, :]
    return out


def kernel(**inputs):
    nc = _get_nc(debug=False)
    in_maps = make_in_maps(inputs)
    res = run_bass_kernel_spmd(nc, in_maps, core_ids=list(range(NC)))
    return _assemble(res.results)

